# revision 11
# baseline (speedup 1.0000x reference)
"""Adaptive-softmax NLL loss kernel for 8 TRN2 NeuronCores.

Strategy (vocab-parallel tensor parallelism + cluster-sorted tokens),
restructured tile-outer for collective overlap:
  - Each core owns a 1/8 column slice of each cluster's vocab range
    (250 + 1000 + 5032 cols) plus the shared remainder column 50256
    (its exp is scaled by 1/8 on every core so the reduced sum is exact).
  - The per-core weight matrix is laid out so every tile's needed columns
    are ONE contiguous span: [c0 250 | heads 3 | c1 1000 | heads 3 |
    c2 5032 | shared 1].  The 3 cluster-head columns are duplicated so
    both light (c0/c1) and heavy (c2) tiles pick them up inside their
    span, with exp segments skipping them.
  - Tokens are host-sorted by cluster id so each 128-token tile is
    (almost always) single-cluster; the output is unscrambled on host.
  - Processing is TILE-OUTER: all weights stay resident in SBUF; each
    tile marches through its span in <=2048-col PSUM chunks (4 banks,
    double buffered).  fp8e4m3 DoubleRow matmuls with k-outer ordering
    (stationary x reused across column sub-blocks).
  - ScalarE computes exp over chunk spans with fused free-dim
    accumulation into 4 fixed slots per tile; mixed-cluster tiles get
    spare slots + a onehot fixup.
  - Target logit x[t] . w[y_t] via indirect-DMA gather of owned weight
    rows (bf16) + multiply/reduce on VectorE, masked by ownership.
  - The cross-core combine is 4 chunked 8KB AllReduces of
    (S_selected, tgt) issued as each 8-tile quarter completes, so only
    the last small AR is exposed in the tail.
  - All Ln/epilogue work is deferred to after the last AR so the Scalar
    FIFO never blocks the exp stream (exactly one Exp->Ln table swap).

Token layout on chip: token t -> (partition p = t % 128, tile i = t // 128).
"""

import os
import sys
from contextlib import ExitStack

import numpy as np

try:
    import concourse  # noqa: F401
except ImportError:  # pragma: no cover
    for _p in ("/opt/trn_rl_repo", "/root/.axon_site/_ro/trn_rl_repo"):
        if os.path.isdir(_p):
            sys.path.insert(0, _p)
            break

import ml_dtypes

import concourse.bass as bass
import concourse.tile as tile
from concourse import bacc, mybir
from concourse.bass_utils import run_bass_kernel_spmd

BF16 = ml_dtypes.bfloat16
FP8 = ml_dtypes.float8_e4m3

VOCAB, HIDDEN = 50257, 1024
NTOK = 4096          # B * L tokens
NCORES = 8
P = 128
NT = NTOK // P       # 32 token tiles
NQ = 4               # AllReduce chunks
QT = NT // NQ        # 8 tiles per chunk
LN8 = float(np.log(8.0))
SX, SW = 16.0, 64.0  # fp8 pre-scales for x and w
INV = 1.0 / (SX * SW)

# --- per-core column geometry (shard-local) ---
C0N, C1N, C2N = 250, 1000, 5032
HA0, HA1 = 250, 253             # cluster-head copy A
C1S, C1E = 253, 1253
HB0, HB1 = 1253, 1256           # cluster-head copy B
C2S, C2E = 1256, 6288
SH = 6288                       # shared remainder column 50256
NCOL = 6289
WPAD = 6304                     # fp8 W free dim padded to %16
SHARD = C0N + C1N + C2N + 1     # 6283 rows in gather table

CLUSTER_SEG = {0: (0, C0N), 1: (C1S, C1E), 2: (C2S, C2E)}
SPAN_LO = {0: 0, 1: HA0, 2: HB0}
SPAN_HI = {0: HA1, 1: C1E, 2: NCOL}
HEADS_AT = {0: (HA0, HA1), 1: (HA0, HA1), 2: (HB0, HB1)}

CHUNK = 2048                    # PSUM window (4 banks)


def _tile_plan(clusters):
    """Static plan for one tile given its sorted cluster list.

    Returns dict with span, chunk windows, per-chunk matmul subs and exp
    segments.  Segments carry (lo, hi, cluster, biased).  Slot ids are
    assigned later (main cluster = highest cluster id present)."""
    lo = SPAN_LO[clusters[0]]
    hi = SPAN_HI[clusters[-1]]
    heads = HEADS_AT[clusters[0]]
    segs = []
    for c in clusters:
        segs.append((CLUSTER_SEG[c][0], CLUSTER_SEG[c][1], c, False))
    if clusters[-1] == 2:
        segs.append((SH, SH + 1, 2, True))
    windows = []
    w = lo
    while w < hi:
        w1 = min(hi, w + CHUNK)
        # bank subs relative to window start
        subs = []
        c = w
        while c < w1:
            nxt = min(w1, c + 512 - (c - w) % 512)
            subs.append((c, nxt))
            c = nxt
        csegs = []
        for (a, b, cl, bia) in segs:
            aa, bb = max(a, w), min(b, w1)
            if aa < bb:
                csegs.append((aa, bb, cl, bia))
        windows.append({"w": (w, w1), "subs": subs, "segs": csegs})
        w = w1
    main = clusters[-1]
    return {"span": (lo, hi), "heads": heads, "windows": windows,
            "clusters": clusters, "main": main}


LAST_RESULT = None  # BassKernelResults of the most recent run (side channel)


def _ensure_ntff_hook():
    """bass_utils' trace path imports antenv.axon_hooks, which the trimmed
    agent image lacks. Register a shim (ctypes NTFF hook if available, else
    None so tracing is skipped gracefully)."""
    try:
        import antenv.axon_hooks  # noqa: F401
        return
    except ImportError:
        pass
    hook = None
    try:
        if "/root/.axon_site" not in sys.path and os.path.isdir("/root/.axon_site"):
            sys.path.append("/root/.axon_site")
        from trn_agent_boot.trn_boot import _ntff_profile_via_ctypes
        hook = _ntff_profile_via_ctypes("/opt/axon/libaxon_pjrt.so")
    except Exception:
        hook = None
    import types

    import antenv

    m = types.ModuleType("antenv.axon_hooks")
    m.get_axon_ntff_profile_hook = lambda _hook=hook: _hook
    m.set_axon_ntff_profile_hook = lambda h: None
    sys.modules["antenv.axon_hooks"] = m
    antenv.axon_hooks = m


def _build_graph(kc, plans, order_proc):
    """Build the SPMD Bass graph. kc = number of 128-row K chunks.
    plans[i] = _tile_plan for original tile i; order_proc = processing
    order of tile indices (heavy c2 first, mixed last)."""
    assert kc % 2 == 0
    k2n = kc // 2
    hp = kc * P
    nc = bacc.Bacc(
        "TRN2",
        target_bir_lowering=False,
        debug=False,
        enable_asserts=False,
        num_devices=NCORES,
    )
    dt = mybir.dt
    fp = dt.float32
    f8 = dt.float8e4
    Exp = mybir.ActivationFunctionType.Exp
    Ln = mybir.ActivationFunctionType.Ln
    Alu = mybir.AluOpType
    X = mybir.AxisListType.X

    XT8 = nc.declare_dram_parameter("xt8", [P, k2n, 2, NTOK], f8, isOutput=False)
    W8 = nc.declare_dram_parameter("w8", [P, k2n, 2, WPAD], f8, isOutput=False)
    xN = nc.declare_dram_parameter("xn", [NTOK, hp], dt.bfloat16, isOutput=False)
    WT = nc.declare_dram_parameter("wt", [SHARD, hp], dt.bfloat16, isOutput=False)
    YI = nc.declare_dram_parameter("yi", [P, NT], dt.int32, isOutput=False)
    OM = nc.declare_dram_parameter("om", [P, NT], fp, isOutput=False)
    OH = nc.declare_dram_parameter("oh", [P, NT * 3], fp, isOutput=False)
    OUT = nc.declare_dram_parameter("out", [P, NT], fp, isOutput=True)

    # spare-slot assignment for mixed tiles: list of (proc_pos, cluster, nsegs)
    nspare = 0
    spare_of = {}  # proc_pos -> (offset, cluster, count)
    for pos, t in enumerate(order_proc):
        pl = plans[t]
        if len(pl["clusters"]) > 1:
            # all clusters except main go to spare slots
            cnt = 0
            for wnd in pl["windows"]:
                for (a, b, cl, bia) in wnd["segs"]:
                    if cl != pl["main"]:
                        cnt += 1
            assert len(pl["clusters"]) == 2, "only 2-cluster mixed tiles supported"
            spare_of[pos] = (nspare, pl["clusters"][0], cnt)
            nspare += cnt
    nspare = max(nspare, 1)

    with ExitStack() as ctx:
        tc = ctx.enter_context(tile.TileContext(nc))
        const = ctx.enter_context(tc.tile_pool(name="const", bufs=1))
        expp = ctx.enter_context(tc.tile_pool(name="expp", bufs=3))
        gpool = ctx.enter_context(tc.tile_pool(name="gpool", bufs=2))
        epi = ctx.enter_context(tc.tile_pool(name="epi", bufs=1))
        dram = ctx.enter_context(tc.tile_pool(name="dram", bufs=1, space="DRAM"))

        # ---- resident inputs ----
        yi_sb = const.tile([P, NT], dt.int32)
        nc.sync.dma_start(out=yi_sb[:], in_=YI[:, :])
        om_sb = const.tile([P, NT], fp)
        nc.sync.dma_start(out=om_sb[:], in_=OM[:, :])
        oh_sb = const.tile([P, NT * 3], fp)
        nc.sync.dma_start(out=oh_sb[:], in_=OH[:, :])

        w8_sb = const.tile([P, k2n, 2, WPAD], f8)
        xT_sb = const.tile([P, k2n, 2, NTOK], f8)

        # Processing starts with tiles needing only W[0:HB0) (the c0/c1
        # region, 1.2MB) while the big c2 region streams in.  W pieces on
        # the Scalar queue, xt8 on Sync, so issue parallelizes.
        nc.scalar.dma_start(out=w8_sb[:, :, :, 0:HB0], in_=W8[:, :, :, 0:HB0])

        def load_xt8_block(b):
            lo, hi = b * 1024, (b + 1) * 1024
            nc.sync.dma_start(out=xT_sb[:, :, :, lo:hi], in_=XT8[:, :, :, lo:hi])

        load_xt8_block(0)
        nc.scalar.dma_start(
            out=w8_sb[:, :, :, HB0:HB0 + 2048], in_=W8[:, :, :, HB0:HB0 + 2048]
        )
        load_xt8_block(1)
        nc.scalar.dma_start(
            out=w8_sb[:, :, :, HB0 + 2048:HB0 + 4096],
            in_=W8[:, :, :, HB0 + 2048:HB0 + 4096],
        )
        load_xt8_block(2)
        load_xt8_block(3)
        nc.scalar.dma_start(
            out=w8_sb[:, :, :, HB0 + 4096:WPAD], in_=W8[:, :, :, HB0 + 4096:WPAD]
        )

        nln8 = const.tile([P, 1], fp)
        nc.vector.memset(nln8[:], -LN8)

        acc = const.tile([P, NT * 4], fp)      # 4 main slots per tile (proc order)
        nc.vector.memset(acc[:], 0.0)
        accs = const.tile([P, nspare], fp)     # spare slots for mixed tiles
        nc.vector.memset(accs[:], 0.0)
        tgt_raw = const.tile([P, NT], fp)      # proc order
        cl_sb = const.tile([P, NT * 3], fp)    # ORIGINAL tile order (for oh)
        # S_all layout: [q, (S, tgt), QT tiles]
        S_all = const.tile([P, NQ, 2, QT], fp)
        R_all = const.tile([P, NQ, 2, QT], fp)

        psum = ctx.enter_context(tc.tile_pool(name="psum", bufs=2, space="PSUM"))
        b_in = [dram.tile([P, 2 * QT], fp, name=f"b_in{q}", tag=f"b_in{q}")
                for q in range(NQ)]
        b_out = [dram.tile([P, 2 * QT], fp, name=f"b_out{q}", tag=f"b_out{q}")
                 for q in range(NQ)]

        def emit_tile(pos, t):
            pl = plans[t]
            slot = 0
            sp_off = spare_of.get(pos, (0, -1, 0))[0]
            heads_done = False
            for wnd in pl["windows"]:
                w0, w1 = wnd["w"]
                ps = psum.tile([P, CHUNK], fp)
                for k in range(k2n):
                    for (a, b) in wnd["subs"]:
                        nc.tensor.matmul(
                            ps[:, a - w0:b - w0],
                            lhsT=xT_sb[:, k, :, t * P:(t + 1) * P],
                            rhs=w8_sb[:, k, :, a:b],
                            start=(k == 0),
                            stop=(k == k2n - 1),
                            perf_mode=mybir.MatmulPerfMode.DoubleRow,
                        )
                if not heads_done and pl["heads"][0] >= w0 and pl["heads"][1] <= w1:
                    h0, h1 = pl["heads"]
                    nc.vector.tensor_scalar_mul(
                        cl_sb[:, pos * 3:(pos + 1) * 3], ps[:, h0 - w0:h1 - w0], INV
                    )
                    heads_done = True
                ex = expp.tile([P, CHUNK], fp, tag="ex")
                for (a, b, cl, bia) in wnd["segs"]:
                    if cl == pl["main"]:
                        acol = acc[:, pos * 4 + slot:pos * 4 + slot + 1]
                        slot += 1
                    else:
                        acol = accs[:, sp_off:sp_off + 1]
                        sp_off += 1
                    nc.scalar.activation(
                        out=ex[:, a - w0:b - w0],
                        in_=ps[:, a - w0:b - w0],
                        func=Exp,
                        bias=(nln8[:] if bia else 0.0),
                        scale=INV,
                        accum_out=acol,
                    )
            assert heads_done and slot <= 4
            # target-logit gather + dot (yi is proc-ordered on host)
            wg = gpool.tile([P, hp], dt.bfloat16, tag="wg", name="wg")
            nc.gpsimd.indirect_dma_start(
                out=wg[:],
                out_offset=None,
                in_=WT[:, :],
                in_offset=bass.IndirectOffsetOnAxis(ap=yi_sb[:, pos:pos + 1], axis=0),
            )
            xr = gpool.tile([P, hp], dt.bfloat16, tag="xr", name="xr")
            nc.sync.dma_start(out=xr[:], in_=xN[t * P:(t + 1) * P, :])
            pr = gpool.tile([P, hp], fp, tag="pr", name="pr")
            nc.vector.tensor_mul(out=pr[:], in0=xr[:], in1=wg[:])
            nc.vector.reduce_sum(out=tgt_raw[:, pos:pos + 1], in_=pr[:], axis=X)

        def emit_quarter(q):
            """Fold acc slots + tgt for proc positions [q*QT, (q+1)*QT) and
            start the chunk's all-reduce."""
            sl = slice(q * QT, (q + 1) * QT)
            acc4 = acc[:].rearrange("p (i s) -> p i s", s=4)
            nc.vector.reduce_sum(out=S_all[:, q, 0, :], in_=acc4[:, sl, :], axis=X)
            # mixed-tile fixup: S = S_main*oh_main + S_spare*oh_spare
            for pos in range(q * QT, (q + 1) * QT):
                if pos not in spare_of:
                    continue
                off, cl_sp, cnt = spare_of[pos]
                pl = plans[order_proc[pos]]
                j = pos - q * QT
                scol = S_all[:, q, 0, j:j + 1]
                nc.vector.tensor_mul(
                    out=scol, in0=scol,
                    in1=oh_sb[:, pos * 3 + pl["main"]:pos * 3 + pl["main"] + 1],
                )
                sps = epi.tile([P, 1], fp, tag=f"sps{pos}", name=f"sps{pos}")
                if cnt > 1:
                    nc.vector.reduce_sum(
                        out=sps[:], in_=accs[:, off:off + cnt], axis=X
                    )
                    src = sps[:]
                else:
                    src = accs[:, off:off + 1]
                nc.vector.tensor_mul(
                    out=sps[:], in0=src,
                    in1=oh_sb[:, pos * 3 + cl_sp:pos * 3 + cl_sp + 1],
                )
                nc.vector.tensor_tensor(out=scol, in0=scol, in1=sps[:], op=Alu.add)
            # tgt partial, ownership-masked (om is proc-ordered on host)
            nc.vector.tensor_mul(
                out=S_all[:, q, 1, :], in0=tgt_raw[:, sl], in1=om_sb[:, sl]
            )
            nc.gpsimd.dma_start(out=b_in[q][:], in_=S_all[:, q, :, :])
            nc.gpsimd.collective_compute(
                "AllReduce",
                Alu.add,
                replica_groups=[list(range(NCORES))],
                ins=[b_in[q].opt()],
                outs=[b_out[q].opt()],
            )

        for pos, t in enumerate(order_proc):
            emit_tile(pos, t)
            if (pos + 1) % QT == 0:
                emit_quarter((pos + 1) // QT - 1)

        # ---- cluster-head path (all tiles, original order) ----
        ecl = epi.tile([P, NT * 3], fp)
        nc.scalar.activation(out=ecl[:], in_=cl_sb[:], func=Exp)
        sum_cl = epi.tile([P, NT], fp)
        nc.vector.reduce_sum(
            out=sum_cl[:], in_=ecl[:].rearrange("p (i c) -> p i c", c=3), axis=X
        )
        clsel_t = epi.tile([P, NT * 3], fp)
        nc.vector.tensor_mul(out=clsel_t[:], in0=cl_sb[:], in1=oh_sb[:])
        cl_sel = epi.tile([P, NT], fp)
        nc.vector.reduce_sum(
            out=cl_sel[:], in_=clsel_t[:].rearrange("p (i c) -> p i c", c=3), axis=X
        )

        # ---- epilogue: everything below waits on collectives ----
        for q in range(NQ):
            nc.sync.dma_start(out=R_all[:, q, :, :], in_=b_out[q][:])
        logS = epi.tile([P, NT], fp)        # proc order
        nc.scalar.activation(
            out=logS[:].rearrange("p (q j) -> p q j", j=QT),
            in_=R_all[:, :, 0, :],
            func=Ln,
        )
        lse_cl = epi.tile([P, NT], fp)      # original order
        nc.scalar.activation(out=lse_cl[:], in_=sum_cl[:], func=Ln)
        # nll[pos] = logS - tgt - cl_sel + lse_cl  (orig-order cols via map)
        v1 = epi.tile([P, NT], fp)          # proc order
        nc.vector.tensor_tensor(
            out=v1[:].rearrange("p (q j) -> p q j", j=QT),
            in0=logS[:].rearrange("p (q j) -> p q j", j=QT),
            in1=R_all[:, :, 1, :],
            op=Alu.subtract,
        )
        v2 = epi.tile([P, NT], fp)          # proc order (cl/oh proc-ordered)
        nc.vector.tensor_sub(out=v2[:], in0=cl_sel[:], in1=lse_cl[:])
        res = epi.tile([P, NT], fp)         # proc order
        nc.vector.tensor_sub(out=res[:], in0=v1[:], in1=v2[:])
        nc.sync.dma_start(out=OUT[:, :], in_=res[:])

    return nc


def _shard_cols(k):
    c0 = np.arange(250 * k, 250 * (k + 1))
    c1 = np.arange(2000 + 1000 * k, 2000 + 1000 * (k + 1))
    c2 = np.arange(10000 + 5032 * k, 10000 + 5032 * (k + 1))
    return c0, c1, c2


def _tok_layout(v):
    """[4096] vector -> [128, 32] with A[p, i] = v[i*128 + p]."""
    return np.ascontiguousarray(v.reshape(NT, P).T)


def _pack_dr(m, width):
    """[hp, width] -> double-row packed [128, hp//256, 2, width] fp8."""
    hp = m.shape[0]
    return np.ascontiguousarray(
        m.reshape(hp // 256, 2, P, width).transpose(2, 0, 1, 3)
    ).astype(FP8)


def kernel(**inputs):
    global LAST_RESULT
    x = np.asarray(inputs["x"], np.float32)
    y = np.asarray(inputs["y"]).astype(np.int64).reshape(-1)
    cw = np.asarray(inputs["cluster_w"], np.float32)
    cb = np.asarray(inputs["cluster_b"], np.float32).reshape(-1)
    lw = np.asarray(inputs["logits_w"], np.float32)
    lb = np.asarray(inputs["logits_b"], np.float32).reshape(-1)

    x_flat = x[:, :-1].reshape(NTOK, HIDDEN)

    # sort tokens by cluster so each 128-token tile is (mostly) one cluster
    c_id_full = (y >= 2000).astype(np.int64) + (y >= 10000).astype(np.int64)
    order = np.argsort(c_id_full, kind="stable")
    x_flat = np.ascontiguousarray(x_flat[order])
    y = y[order]
    c_id = c_id_full[order]

    nz_bias = bool(np.any(cb)) or bool(np.any(lb))
    kc = HIDDEN // P + (2 if nz_bias else 0)
    hp = kc * P
    if nz_bias:
        xa = np.zeros((NTOK, hp), np.float32)
        xa[:, :HIDDEN] = x_flat
        xa[:, HIDDEN] = 1.0
        lwa = np.zeros((hp, VOCAB), np.float32)
        lwa[:HIDDEN] = lw
        lwa[HIDDEN] = lb
        cwa = np.zeros((hp, 3), np.float32)
        cwa[:HIDDEN] = cw
        cwa[HIDDEN] = cb
        x_flat, lw, cw = xa, lwa, cwa

    xT = np.ascontiguousarray(x_flat.T)  # [hp, NTOK]
    xt8 = _pack_dr(xT * SX, NTOK)
    xN_bf = x_flat.astype(BF16)

    # per-tile cluster lists + processing order: pure c2, light, mixed last
    tiles_cl = []
    for i in range(NT):
        tiles_cl.append(sorted(set(c_id[i * P:(i + 1) * P].tolist())))
    # tiles with no c2 need only W[0:HB0): process them first so compute
    # starts while the big c2 weight region streams in; c2 tiles follow so
    # each AR chunk carries >=28us of compute (ARs serialize on the CC ring).
    nc2 = [i for i in range(NT) if 2 not in tiles_cl[i]]
    c2t = [i for i in range(NT) if 2 in tiles_cl[i]]
    order_proc = nc2 + c2t
    assert len(order_proc) == NT
    plans = [_tile_plan(cl) for cl in tiles_cl]

    # onehot over clusters, [128, 32*3] with c contiguous, PROC tile order
    oh = np.zeros((NTOK, 3), np.float32)
    oh[np.arange(NTOK), c_id] = 1.0
    oh = oh.reshape(NT, P, 3)[order_proc]
    oh = np.ascontiguousarray(oh.transpose(1, 0, 2).reshape(P, NT * 3))

    in_maps = []
    for k in range(NCORES):
        c0, c1, c2 = _shard_cols(k)
        wpadded = np.zeros((hp, WPAD), np.float32)
        wpadded[:, 0:C0N] = lw[:, c0]
        wpadded[:, HA0:HA1] = cw
        wpadded[:, C1S:C1E] = lw[:, c1]
        wpadded[:, HB0:HB1] = cw
        wpadded[:, C2S:C2E] = lw[:, c2]
        wpadded[:, SH] = lw[:, VOCAB - 1]  # shared col (exp biased by -ln8)
        w8 = _pack_dr(wpadded * SW, WPAD)

        # gather table rows: [c0 | c1 | c2 | shared]
        w_sh = np.concatenate(
            [lw[:, c0], lw[:, c1], lw[:, c2], lw[:, VOCAB - 1:VOCAB]], axis=1)
        wt_bf = np.ascontiguousarray(w_sh.T).astype(BF16)

        loc = np.zeros(NTOK, np.int64)
        r0 = (y >= 250 * k) & (y < 250 * (k + 1))
        loc[r0] = y[r0] - 250 * k
        r1 = (y >= 2000 + 1000 * k) & (y < 2000 + 1000 * (k + 1))
        loc[r1] = 250 + y[r1] - (2000 + 1000 * k)
        r2 = (y >= 10000 + 5032 * k) & (y < 10000 + 5032 * (k + 1))
        loc[r2] = 1250 + y[r2] - (10000 + 5032 * k)
        own = r0 | r1 | r2
        if k == NCORES - 1:
            r3 = y == VOCAB - 1
            own = own | r3
            loc[r3] = SHARD - 1

        in_maps.append(
            {
                "xt8": xt8,
                "w8": w8,
                "xn": xN_bf,
                "wt": wt_bf,
                # yi/om in PROC tile order to match on-chip indexing
                "yi": np.ascontiguousarray(
                    _tok_layout(loc)[:, order_proc]).astype(np.int32),
                "om": np.ascontiguousarray(
                    _tok_layout(own.astype(np.float32))[:, order_proc]),
                "oh": oh,
            }
        )

    _ensure_ntff_hook()
    nc = _build_graph(kc, plans, order_proc)
    if not nc.is_finalized():
        nc.finalize()
    result = run_bass_kernel_spmd(nc, in_maps, core_ids=list(range(NCORES)))
    LAST_RESULT = result
    out = np.asarray(result.results[0]["out"], np.float32)  # [128, 32] proc order
    nll_sorted = np.empty(NTOK, np.float32)
    for pos, t in enumerate(order_proc):
        nll_sorted[t * P:(t + 1) * P] = out[:, pos]
    nll = np.empty(NTOK, np.float32)
    nll[order] = nll_sorted
    return nll
